# revision 1
# baseline (speedup 1.0000x reference)
"""Enformer-style relative-position attention (nn_Attention_27925877358942) for
8 Trainium2 NeuronCores.

Contract: kernel(**inputs) takes the FULL unsharded inputs (keys as in
setup_inputs()) and returns the full [1, 4096, 1536] float32 output.

Sharding: one head per core (8 heads / 8 cores). Host precomputes the
deterministic positional-feature table and x^T in fp16, slices per-head
weights, runs the SPMD Bass kernel via run_bass_kernel_spmd, and sums the
per-head output projections (+ b_out).

Device pipeline per core (head h), N=4096, d=64:
  - q^T,k^T (fp16, [64,N]) and v ([N,65] with ones col) projections on PE
  - r^T = (pos @ Wrelk_h)^T from the positional table
  - per query tile I: window logits em = exp((q+bp) . r[t0:t0+4223]) (ACT, bf16)
  - relative_shift via DRAM roundtrip: sheared strided read
      shr[di, j] = em[di, 127-di+j] (partition step = rowpitch-1 elements)
  - content logits transposed C^T = k_J . q_I (PE), exp on ACT
  - pT = exp(C^T) * transpose(shr) (PE transpose + DVE multiply, bf16)
  - O = pT.T @ [v|1] accumulated in PSUM; epilogue normalizes by the row sums
    and applies the per-head slice of W_out; host sums partials over heads.

This walrus build accepts at most ONE sync wait per instruction, so after
Tile scheduling every multi-wait instruction is split by inserting
wait-carrying NoOps just before it on the same engine (split_multi_waits),
and the Tile tail drain is built with the same constraint.
"""


_DRAIN_PATCHED = [False]


def _patch_tile_drain():
    if _DRAIN_PATCHED[0]:
        return
    _DRAIN_PATCHED[0] = True
    import concourse.tile as tile_mod
    from concourse.vector_clock import ScopedClock

    MAX_WAITS = 1

    def _drain_and_barrier(self, tick_clock, wait_clock):
        nc = self.nc
        drain_inst = nc.sync.drain()
        wait_clock.add_sem_waits(drain_inst.ins, ScopedClock({None: tick_clock.global_clock}))
        si = drain_inst.ins.sync_info
        waits = list(si.on_wait) if si is not None and si.on_wait else []
        if len(waits) > MAX_WAITS:
            si.on_wait = waits[:MAX_WAITS]
            rest = waits[MAX_WAITS:]
            import concourse.mybir as _mb
            for i in range(0, len(rest), MAX_WAITS):
                extra = nc.sync.drain()
                esi = extra.ins.sync_info
                if esi is None:
                    extra.ins.sync_info = _mb.SyncInfo(on_wait=rest[i:i + MAX_WAITS], on_update=[])
                else:
                    esi.on_wait = rest[i:i + MAX_WAITS]
        nc.all_engine_barrier()
        assert self.sems is not None
        popped = nc._tile_sem_poison_stack.pop()
        assert popped is self._sem_poison
        nc.clear_and_free_semaphores(list(self.sems.allocated().values()))
        nc.all_engine_barrier()

    tile_mod.TileContext._drain_and_barrier = _drain_and_barrier


def split_multi_waits(nc):
    """This walrus build allows at most ONE sync wait per instruction.
    Move extra waits onto InstNoOp carriers inserted just before, on the
    same engine queue (sequencers execute in order, so semantics hold)."""
    import concourse.mybir as mb
    n_split = 0
    for fn in nc.m.functions:
        for bb in fn.blocks:
            insts = list(bb.instructions)
            out = []
            for inst in insts:
                si = inst.sync_info
                waits = list(si.on_wait) if si is not None and si.on_wait else []
                if len(waits) > 1:
                    for w in waits[:-1]:
                        n_split += 1
                        nop = mb.InstNoOp(
                            name=f"waitsplit-{n_split}",
                            engine=inst.engine,
                            sync_info=mb.SyncInfo(on_wait=[w], on_update=[]),
                        )
                        out.append(nop)
                    si.on_wait = [waits[-1]]
                out.append(inst)
            if len(out) != len(insts):
                bb.instructions[:] = out
    return n_split


import math
from contextlib import ExitStack

import numpy as np

import concourse.bass as bass
import concourse.tile as tile
from concourse import mybir
from concourse.bass import ts, ds
from concourse.masks import make_identity

F32 = mybir.dt.float32
BF16 = mybir.dt.bfloat16
FP16 = mybir.dt.float16
AF = mybir.ActivationFunctionType

DIM = 1536
H = 8
D = 64


def build(N, split_waits=True):
    Q = N // 128           # query tiles
    NJ = N // 128          # key tiles
    PW = 2 * N             # padded positional width (2N-1 real cols + 1 pad)
    WN = N + 128           # rel window width per q-tile (incl. 1 pad col)
    KD = DIM // 128        # contraction tiles for projections

    nc = bass.Bass("TRN2", target_bir_lowering=False, debug=False)

    xT_d = nc.dram_tensor("xT", [DIM, N], FP16, kind="ExternalInput")
    posT_d = nc.dram_tensor("posT", [192, PW], FP16, kind="ExternalInput")
    wq_d = nc.dram_tensor("wq", [DIM, D], FP16, kind="ExternalInput")
    wk_d = nc.dram_tensor("wk", [DIM, D], FP16, kind="ExternalInput")
    wv_d = nc.dram_tensor("wv", [DIM, D], FP16, kind="ExternalInput")
    wrk_d = nc.dram_tensor("wrk", [192, D], FP16, kind="ExternalInput")
    wo_d = nc.dram_tensor("wo", [D, DIM], BF16, kind="ExternalInput")
    bc_d = nc.dram_tensor("bc", [D, 1], F32, kind="ExternalInput")
    bp_d = nc.dram_tensor("bp", [D, 1], F32, kind="ExternalInput")
    out_d = nc.dram_tensor("out", [N, DIM], FP16, kind="ExternalOutput")
    em_d = nc.dram_tensor("em_scratch", [Q * 128, WN], BF16, kind="Internal")

    scale = D ** -0.5

    with tile.TileContext(nc) as tc, ExitStack() as ctx:
        consts = ctx.enter_context(tc.tile_pool(name="consts", bufs=1))
        persist = ctx.enter_context(tc.tile_pool(name="persist", bufs=1))

        # ---- constants ----
        ident = consts.tile([128, 128], BF16, tag="ident")
        make_identity(nc, ident[:])
        bc_sb = consts.tile([D, 1], F32, tag="bc")
        nc.sync.dma_start(out=bc_sb[:], in_=bc_d.ap())
        bp_sb = consts.tile([D, 1], F32, tag="bp")
        nc.sync.dma_start(out=bp_sb[:], in_=bp_d.ap())
        wo_sb = consts.tile([D, DIM], BF16, tag="wo")
        nc.sync.dma_start(out=wo_sb[:], in_=wo_d.ap())

        wqk_sb = consts.tile([128, KD, 2 * D], FP16, tag="wqk")
        wv_sb = consts.tile([128, KD, D], FP16, tag="wv")
        nc.sync.dma_start(out=wqk_sb[:, :, 0:D],
                          in_=wq_d.ap().rearrange("(t p) c -> p t c", p=128))
        nc.sync.dma_start(out=wqk_sb[:, :, D:2 * D],
                          in_=wk_d.ap().rearrange("(t p) c -> p t c", p=128))
        nc.sync.dma_start(out=wv_sb[:],
                          in_=wv_d.ap().rearrange("(t p) c -> p t c", p=128))
        wrk_sb = consts.tile([96, 2, D], FP16, tag="wrk")
        for u in range(2):
            nc.sync.dma_start(out=wrk_sb[:, u, :], in_=wrk_d[ts(u, 96), :])

        # ---- persistent activations ----
        qcT = persist.tile([D, N], FP16, tag="qcT")
        qpT = persist.tile([D, N], FP16, tag="qpT")
        kT = persist.tile([D, N], FP16, tag="kT")
        rT = persist.tile([D, PW], FP16, tag="rT")
        vext = persist.tile([128, NJ * (D + 1)], BF16, tag="vext")

        # ---- phases 1-2: rel-k table + projections ----
        with tc.tile_pool(name="stream", bufs=1) as stream, \
             tc.tile_pool(name="prep_psum", bufs=2, space="PSUM") as prep_psum:
            # rel-k table first: independent of x, runs during the xT load
            pall = stream.tile([96, 2, PW], FP16, tag="pall")
            nc.sync.dma_start(out=pall[:, 0, :], in_=posT_d[0:96, :])
            nc.sync.dma_start(out=pall[:, 1, :], in_=posT_d[96:192, :])
            for rc in reversed(range(PW // 512)):
                c0 = rc * 512
                pc = pall[:, :, ds(c0, 512)]
                ps_r = prep_psum.tile([D, 512], F32, tag="ps_qk")
                for u in range(2):
                    nc.tensor.matmul(
                        ps_r[:], wrk_sb[:, u, :], pc[:, u, :],
                        start=(u == 0), stop=(u == 1),
                    )
                nc.scalar.copy(out=rT[:, ds(c0, 512)], in_=ps_r[:])

            xall = stream.tile([128, KD, N], FP16, tag="xall")
            xT_v = xT_d.ap().rearrange("(t p) n -> p t n", p=128)
            for oct_ in range(8):
                h0 = oct_ * (N // 8)
                nc.sync.dma_start(
                    out=xall[:, :, ds(h0, N // 8)],
                    in_=xT_v[:, :, ds(h0, N // 8)],
                )
            for ic in range(N // 512):
                i0 = ic * 512
                xc = xall[:, :, ds(i0, 512)]
                ps_qk = prep_psum.tile([128, 512], F32, tag="ps_qk")
                for kd in range(KD):
                    nc.tensor.matmul(
                        ps_qk[:], wqk_sb[:, kd, :], xc[:, kd, :],
                        start=(kd == 0), stop=(kd == KD - 1),
                    )
                nc.scalar.activation(
                    out=qcT[:, ds(i0, 512)], in_=ps_qk[0:D, :], func=AF.Identity,
                    bias=bc_sb[:], scale=scale,
                )
                nc.scalar.activation(
                    out=qpT[:, ds(i0, 512)], in_=ps_qk[0:D, :], func=AF.Identity,
                    bias=bp_sb[:], scale=scale,
                )
                nc.scalar.copy(out=kT[:, ds(i0, 512)], in_=ps_qk[D:2 * D, :])
                for isb in range(4):
                    J = ic * 4 + isb
                    ps_v = prep_psum.tile([128, D], F32, tag="ps_v")
                    for kd in range(KD):
                        nc.tensor.matmul(
                            ps_v[:], xc[:, kd, ts(isb, 128)], wv_sb[:, kd, :],
                            start=(kd == 0), stop=(kd == KD - 1),
                        )
                    nc.scalar.copy(out=vext[:, ds(J * (D + 1), D)], in_=ps_v[:])
                    nc.vector.memset(vext[:, ds(J * (D + 1) + D, 1)], 1.0)

        # ---- phase 3: main loop, q-tiles in pairs ----
        work = ctx.enter_context(tc.tile_pool(name="work", bufs=2))
        wshear = ctx.enter_context(tc.tile_pool(name="wshear", bufs=4))
        sm = ctx.enter_context(tc.tile_pool(name="sm", bufs=3))
        ppool_m = ctx.enter_context(tc.tile_pool(name="ppool_m", bufs=2, space="PSUM"))
        ppool_ct = ctx.enter_context(tc.tile_pool(name="ppool_ct", bufs=2, space="PSUM"))
        ppool_st = ctx.enter_context(tc.tile_pool(name="ppool_st", bufs=1, space="PSUM"))
        ppool_epi = ctx.enter_context(tc.tile_pool(name="ppool_epi", bufs=1, space="PSUM"))

        for g in range(Q // 2):
            i0g = g * 256
            shr_pair = []
            for q in range(2):
                I = 2 * g + q
                i0 = I * 128
                t0 = N - 1 - i0 - 127

                em_sb = wshear.tile([128, WN], BF16, tag="em")
                n_full = (WN - 128) // 1024
                chunks = [(c * 1024, 1024) for c in range(n_full)]
                chunks.append((n_full * 1024, WN - 1 - n_full * 1024))
                for (c0, cw) in chunks:
                    ps = ppool_m.tile([128, 1024], F32, tag="ps_m")
                    for s0 in range(0, cw, 512):
                        sw = min(512, cw - s0)
                        nc.tensor.matmul(
                            ps[:, ds(s0, sw)], qpT[:, ds(i0, 128)],
                            rT[:, ds(t0 + c0 + s0, sw)],
                            start=True, stop=True,
                        )
                    nc.scalar.activation(
                        out=em_sb[:, ds(c0, cw)], in_=ps[:, 0:cw], func=AF.Exp,
                    )
                nc.sync.dma_start(out=em_d[ds(i0, 128), 0:WN - 1],
                                  in_=em_sb[:, 0:WN - 1])
                shr_sb = wshear.tile([128, N], BF16, tag="shr")
                shear_ap = bass.AP(em_d, i0 * WN + 127, [[WN - 1, 128], [1, N]])
                nc.sync.dma_start(out=shr_sb[:], in_=shear_ap)
                shr_pair.append(shr_sb)

            # content logits transposed: ecT[dj, J*256 + q*128 + di]
            ecT_sb = work.tile([128, NJ * 256], BF16, tag="ecT")
            for Jg in range(NJ // 2):
                ps = ppool_ct.tile([128, 512], F32, tag="ps_ct")
                for u in range(2):
                    J = Jg * 2 + u
                    nc.tensor.matmul(
                        ps[:, ts(u, 256)], kT[:, ts(J, 128)], qcT[:, ds(i0g, 256)],
                        start=True, stop=True,
                    )
                nc.scalar.activation(
                    out=ecT_sb[:, ds(Jg * 512, 512)], in_=ps[:], func=AF.Exp,
                )

            # pT = ecT * shr^T
            pT_sb = work.tile([128, NJ * 256], BF16, tag="pT")
            for Jg in range(NJ // 4):
                ps_t = ppool_st.tile([128, 1024], BF16, tag="ps_st")
                for u in range(4):
                    J = Jg * 4 + u
                    for q in range(2):
                        nc.tensor.transpose(
                            ps_t[:, ds(u * 256 + q * 128, 128)],
                            shr_pair[q][:, ts(J, 128)], ident[:],
                        )
                nc.vector.tensor_mul(
                    pT_sb[:, ds(Jg * 1024, 1024)], ecT_sb[:, ds(Jg * 1024, 1024)], ps_t[:]
                )

            # PV + epilogue per q-tile
            for q in range(2):
                i0 = i0g + q * 128
                ps_o = ppool_epi.tile([128, D + 1], F32, tag="ps_epi")
                for J in range(NJ):
                    nc.tensor.matmul(
                        ps_o[:], pT_sb[:, ds(J * 256 + q * 128, 128)],
                        vext[:, ds(J * (D + 1), D + 1)],
                        start=(J == 0), stop=(J == NJ - 1),
                    )
                rc_sb = sm.tile([128, 1], F32, tag="rc")
                nc.vector.reciprocal(out=rc_sb[:], in_=ps_o[:, D:D + 1])
                o_sb = sm.tile([128, D], BF16, tag="o")
                nc.vector.tensor_copy(o_sb[:], ps_o[:, 0:D])
                ps_ot = ppool_epi.tile([D, 128], BF16, tag="ps_epi")
                nc.tensor.transpose(ps_ot[:], o_sb[:], ident[:])
                otT_sb = sm.tile([D, 128], BF16, tag="otT")
                nc.vector.tensor_copy(otT_sb[:], ps_ot[:])
                out_sb = work.tile([128, DIM], FP16, tag="out")
                for w in range(DIM // 512):
                    ps_op = ppool_epi.tile([128, 512], F32, tag="ps_epi")
                    nc.tensor.matmul(
                        ps_op[:], otT_sb[:], wo_sb[:, ts(w, 512)],
                        start=True, stop=True,
                    )
                    nc.vector.tensor_scalar_mul(
                        out_sb[:, ts(w, 512)], ps_op[:], rc_sb[:]
                    )
                nc.sync.dma_start(out=out_d[ds(i0, 128), :], in_=out_sb[:])

    if split_waits:
        _patch_tile_drain()
        split_multi_waits(nc)
    return nc


# ---------------- host side ----------------

def get_positional_embed_np(seq_len, feature_size):
    distances = np.arange(-seq_len + 1, seq_len)
    nb = feature_size // 2
    pow_rate = math.exp(math.log(seq_len + 1) / nb)
    center_widths = np.power(np.float32(pow_rate), np.arange(1, nb + 1, dtype=np.float32)) - 1.0
    emb = (center_widths[None, :] > np.abs(distances)[:, None]).astype(np.float32)
    signed = np.sign(distances).astype(np.float32)[:, None] * emb
    return np.concatenate([emb, signed], axis=-1)  # [2n-1, F]


def make_in_maps(x, W_q, W_k, W_v, W_rel_k, W_out, rel_content_bias, rel_pos_bias):
    B, N, _ = np.asarray(x).shape
    PW = 2 * N
    f16 = np.float16
    import ml_dtypes
    bf16 = ml_dtypes.bfloat16
    xT = np.ascontiguousarray(np.asarray(x[0], np.float32).T).astype(f16)
    pos = get_positional_embed_np(N, np.asarray(W_rel_k).shape[0])
    posT = np.zeros((192, PW), np.float32)
    posT[:, : 2 * N - 1] = pos.T
    posT = posT.astype(f16)
    in_maps = []
    for h in range(H):
        sl = slice(h * D, (h + 1) * D)
        in_maps.append({
            "xT": xT,
            "posT": posT,
            "wq": np.ascontiguousarray(np.asarray(W_q)[:, sl]).astype(f16),
            "wk": np.ascontiguousarray(np.asarray(W_k)[:, sl]).astype(f16),
            "wv": np.ascontiguousarray(np.asarray(W_v)[:, sl]).astype(f16),
            "wrk": np.ascontiguousarray(np.asarray(W_rel_k)[:, sl]).astype(f16),
            "wo": np.ascontiguousarray(np.asarray(W_out)[sl, :]).astype(bf16),
            "bc": np.ascontiguousarray(
                np.asarray(rel_content_bias, np.float32)[0, h, 0, :].reshape(D, 1)),
            "bp": np.ascontiguousarray(
                np.asarray(rel_pos_bias, np.float32)[0, h, 0, :].reshape(D, 1)),
        })
    return in_maps


def combine_outputs(results, b_out):
    acc = None
    for r in results:
        p = r["out"].astype(np.float32)
        acc = p if acc is None else acc + p
    acc = acc + np.asarray(b_out, np.float32)[None, :]
    return acc[None]  # [1, N, DIM]


# ---------------- entry point ----------------

_NC_CACHE = {}


def kernel(x, W_q, W_k, W_v, W_rel_k, W_out, b_out,
           rel_content_bias, rel_pos_bias):
    """Full-input entry: shards per head across 8 NeuronCores, returns the
    full [1, N, 1536] float32 output."""
    from concourse import bass_utils

    x = np.asarray(x)
    N = x.shape[1]
    if N not in _NC_CACHE:
        _NC_CACHE[N] = build(N)
    nc = _NC_CACHE[N]
    in_maps = make_in_maps(x, W_q, W_k, W_v, W_rel_k, W_out,
                           rel_content_bias, rel_pos_bias)
    res = bass_utils.run_bass_kernel_spmd(nc, in_maps, core_ids=list(range(H)))
    return combine_outputs(res.results, b_out).astype(np.float32)



# revision 6
# speedup vs baseline: 1.0809x; 1.0809x over previous
"""Enformer-style relative-position attention (nn_Attention_27925877358942) for
8 Trainium2 NeuronCores.

Contract: kernel(**inputs) takes the FULL unsharded inputs (keys as in
setup_inputs()) and returns the full [1, 4096, 1536] float32 output.

Sharding: one head per core (8 heads / 8 cores). Host precomputes the
deterministic positional-feature table and x^T in fp16, slices per-head
weights, runs the SPMD Bass kernel via run_bass_kernel_spmd, and sums the
per-head output projections (+ b_out).

Device pipeline per core (head h), N=4096, d=64:
  - q^T,k^T (fp16, [64,N]) and v ([N,65] with ones col) projections on PE
  - r^T = (pos @ Wrelk_h)^T from the positional table
  - per query tile I: window logits em = exp((q+bp) . r[t0:t0+4223]) (ACT, bf16)
  - relative_shift via DRAM roundtrip: sheared strided read
      shr[di, j] = em[di, 127-di+j] (partition step = rowpitch-1 elements)
  - content logits transposed C^T = k_J . q_I (PE), exp on ACT
  - pT = exp(C^T) * transpose(shr) (PE transpose + DVE multiply, bf16)
  - O = pT.T @ [v|1] accumulated in PSUM; epilogue normalizes by the row sums
    and applies the per-head slice of W_out; host sums partials over heads.

This walrus build accepts at most ONE sync wait per instruction, so after
Tile scheduling every multi-wait instruction is split by inserting
wait-carrying NoOps just before it on the same engine (split_multi_waits),
and the Tile tail drain is built with the same constraint.
"""


_DRAIN_PATCHED = [False]


def _patch_tile_drain():
    if _DRAIN_PATCHED[0]:
        return
    _DRAIN_PATCHED[0] = True
    import concourse.tile as tile_mod
    from concourse.vector_clock import ScopedClock

    MAX_WAITS = 1

    def _drain_and_barrier(self, tick_clock, wait_clock):
        nc = self.nc
        drain_inst = nc.sync.drain()
        wait_clock.add_sem_waits(drain_inst.ins, ScopedClock({None: tick_clock.global_clock}))
        si = drain_inst.ins.sync_info
        waits = list(si.on_wait) if si is not None and si.on_wait else []
        if len(waits) > MAX_WAITS:
            si.on_wait = waits[:MAX_WAITS]
            rest = waits[MAX_WAITS:]
            import concourse.mybir as _mb
            for i in range(0, len(rest), MAX_WAITS):
                extra = nc.sync.drain()
                esi = extra.ins.sync_info
                if esi is None:
                    extra.ins.sync_info = _mb.SyncInfo(on_wait=rest[i:i + MAX_WAITS], on_update=[])
                else:
                    esi.on_wait = rest[i:i + MAX_WAITS]
        nc.all_engine_barrier()
        assert self.sems is not None
        popped = nc._tile_sem_poison_stack.pop()
        assert popped is self._sem_poison
        nc.clear_and_free_semaphores(list(self.sems.allocated().values()))
        nc.all_engine_barrier()

    tile_mod.TileContext._drain_and_barrier = _drain_and_barrier


def split_multi_waits(nc):
    """This walrus build allows at most ONE sync wait per instruction.
    Move extra waits onto InstNoOp carriers inserted just before, on the
    same engine queue (sequencers execute in order, so semantics hold)."""
    import concourse.mybir as mb
    n_split = 0
    for fn in nc.m.functions:
        for bb in fn.blocks:
            insts = list(bb.instructions)
            out = []
            for inst in insts:
                si = inst.sync_info
                waits = list(si.on_wait) if si is not None and si.on_wait else []
                if len(waits) > 1:
                    for w in waits[:-1]:
                        n_split += 1
                        nop = mb.InstNoOp(
                            name=f"waitsplit-{n_split}",
                            engine=inst.engine,
                            sync_info=mb.SyncInfo(on_wait=[w], on_update=[]),
                        )
                        out.append(nop)
                    si.on_wait = [waits[-1]]
                out.append(inst)
            if len(out) != len(insts):
                bb.instructions[:] = out
    return n_split


import math
from contextlib import ExitStack

import numpy as np

import concourse.bass as bass
import concourse.tile as tile
from concourse import mybir
from concourse.bass import ts, ds
from concourse.masks import make_identity

F32 = mybir.dt.float32
BF16 = mybir.dt.bfloat16
FP16 = mybir.dt.float16
U16 = mybir.dt.uint16
AF = mybir.ActivationFunctionType

DIM = 1536
H = 8
D = 64
NSEG = None  # set by build() from the static positional segmentation


def pos_segments(N):
    """Static segmentation of the 2N-1 relative positions: the central-mask
    features are piecewise constant in the distance, so the rel-k table has
    only ~157 distinct columns. Returns (seg_starts, seg_of)."""
    pos = get_positional_embed_np(N, 192)
    diffs = np.any(pos[1:] != pos[:-1], axis=1)
    seg_starts = np.concatenate([[0], np.nonzero(diffs)[0] + 1]).astype(np.int64)
    seg_of = np.zeros(2 * N - 1, np.int64)
    seg_of[seg_starts] = 1
    seg_of = np.cumsum(seg_of) - 1
    return seg_starts, seg_of


def build(N, split_waits=True, ic_chunk=1024):
    Q = N // 128           # query tiles
    NJ = N // 128          # key tiles
    WN = N + 128           # rel window width per q-tile (incl. 1 pad col)
    KD = DIM // 128        # contraction tiles for projections
    S = len(pos_segments(N)[0])  # distinct rel-k columns (157 for N=4096)
    WCOLS = (2 * N - 1 + WN + 15) // 16  # wrapped master index cols

    nc = bass.Bass("TRN2", target_bir_lowering=False, debug=False)

    xT_d = nc.dram_tensor("xT", [DIM, N], FP16, kind="ExternalInput")
    pseg_d = nc.dram_tensor("pseg", [192, S], FP16, kind="ExternalInput")
    widx_d = nc.dram_tensor("widx", [128, WCOLS], U16, kind="ExternalInput")
    wq_d = nc.dram_tensor("wq", [DIM, D], FP16, kind="ExternalInput")
    wk_d = nc.dram_tensor("wk", [DIM, D], FP16, kind="ExternalInput")
    wv_d = nc.dram_tensor("wv", [DIM, D], FP16, kind="ExternalInput")
    wrk_d = nc.dram_tensor("wrk", [192, D], FP16, kind="ExternalInput")
    wo_d = nc.dram_tensor("wo", [D, DIM], BF16, kind="ExternalInput")
    bc_d = nc.dram_tensor("bc", [D, 1], F32, kind="ExternalInput")
    bp_d = nc.dram_tensor("bp", [D, 1], F32, kind="ExternalInput")
    out_d = nc.dram_tensor("out", [N, DIM], FP16, kind="ExternalOutput")
    em_d = nc.dram_tensor("em_scratch", [Q * 128, WN], BF16, kind="Internal")

    scale = D ** -0.5

    with tile.TileContext(nc) as tc, ExitStack() as ctx:
        consts = ctx.enter_context(tc.tile_pool(name="consts", bufs=1))
        persist = ctx.enter_context(tc.tile_pool(name="persist", bufs=1))

        # ---- constants ----
        ident = consts.tile([128, 128], BF16, tag="ident")
        make_identity(nc, ident[:])
        bc_sb = consts.tile([D, 1], F32, tag="bc")
        nc.sync.dma_start(out=bc_sb[:], in_=bc_d.ap())
        bp_sb = consts.tile([D, 1], F32, tag="bp")
        nc.sync.dma_start(out=bp_sb[:], in_=bp_d.ap())
        wo_sb = consts.tile([D, DIM], BF16, tag="wo")
        nc.sync.dma_start(out=wo_sb[:], in_=wo_d.ap())

        wqk_sb = consts.tile([128, KD, 2 * D], FP16, tag="wqk")
        wv_sb = consts.tile([128, KD, D], FP16, tag="wv")
        nc.sync.dma_start(out=wqk_sb[:, :, 0:D],
                          in_=wq_d.ap().rearrange("(t p) c -> p t c", p=128))
        nc.sync.dma_start(out=wqk_sb[:, :, D:2 * D],
                          in_=wk_d.ap().rearrange("(t p) c -> p t c", p=128))
        nc.sync.dma_start(out=wv_sb[:],
                          in_=wv_d.ap().rearrange("(t p) c -> p t c", p=128))
        wrk_sb = consts.tile([96, 2, D], FP16, tag="wrk")
        for u in range(2):
            nc.sync.dma_start(out=wrk_sb[:, u, :], in_=wrk_d[ts(u, 96), :])

        # ---- persistent activations ----
        qcT = persist.tile([D, N], FP16, tag="qcT")
        qpT = persist.tile([D, N], FP16, tag="qpT")
        kT = persist.tile([D, N], FP16, tag="kT")
        rsT = persist.tile([D, S], FP16, tag="rsT")
        widx_sb = persist.tile([128, WCOLS], U16, tag="widx")
        vext = persist.tile([128, NJ * (D + 1)], BF16, tag="vext")

        nc.sync.dma_start(out=widx_sb[:], in_=widx_d.ap())

        # ---- phases 1-2: rel-k table (distinct cols only) + projections ----
        with tc.tile_pool(name="stream", bufs=1) as stream, \
             tc.tile_pool(name="prep_psum", bufs=2, space="PSUM") as prep_psum:
            pall = stream.tile([96, 2, S], FP16, tag="pall")
            nc.sync.dma_start(out=pall[:, 0, :], in_=pseg_d[0:96, :])
            nc.sync.dma_start(out=pall[:, 1, :], in_=pseg_d[96:192, :])
            ps_r = prep_psum.tile([D, S], F32, tag="ps_qk")
            for u in range(2):
                nc.tensor.matmul(
                    ps_r[:], wrk_sb[:, u, :], pall[:, u, :],
                    start=(u == 0), stop=(u == 1),
                )
            nc.scalar.copy(out=rsT[:], in_=ps_r[:])

            xall = stream.tile([128, KD, N], FP16, tag="xall")
            xT_v = xT_d.ap().rearrange("(t p) n -> p t n", p=128)
            for oct_ in range(8):
                h0 = oct_ * (N // 8)
                nc.sync.dma_start(
                    out=xall[:, :, ds(h0, N // 8)],
                    in_=xT_v[:, :, ds(h0, N // 8)],
                )
            for ic in range(N // 512):
                i0 = ic * 512
                xc = xall[:, :, ds(i0, 512)]
                ps_qk = prep_psum.tile([128, 512], F32, tag="ps_qk")
                for kd in range(KD):
                    nc.tensor.matmul(
                        ps_qk[:], wqk_sb[:, kd, :], xc[:, kd, :],
                        start=(kd == 0), stop=(kd == KD - 1),
                    )
                nc.scalar.activation(
                    out=qcT[:, ds(i0, 512)], in_=ps_qk[0:D, :], func=AF.Identity,
                    bias=bc_sb[:], scale=scale,
                )
                nc.scalar.activation(
                    out=qpT[:, ds(i0, 512)], in_=ps_qk[0:D, :], func=AF.Identity,
                    bias=bp_sb[:], scale=scale,
                )
                nc.scalar.copy(out=kT[:, ds(i0, 512)], in_=ps_qk[D:2 * D, :])
                for isb in range(4):
                    J = ic * 4 + isb
                    ps_v = prep_psum.tile([128, D], F32, tag="ps_v")
                    for kd in range(KD):
                        nc.tensor.matmul(
                            ps_v[:], xc[:, kd, ts(isb, 128)], wv_sb[:, kd, :],
                            start=(kd == 0), stop=(kd == KD - 1),
                        )
                    nc.scalar.copy(out=vext[:, ds(J * (D + 1), D)], in_=ps_v[:])
                    nc.vector.memset(vext[:, ds(J * (D + 1) + D, 1)], 1.0)

        # ---- phase 3: main loop, q-tiles in pairs ----
        work = ctx.enter_context(tc.tile_pool(name="work", bufs=2))
        wshear = ctx.enter_context(tc.tile_pool(name="wshear", bufs=4))
        sm = ctx.enter_context(tc.tile_pool(name="sm", bufs=3))
        ppool_m = ctx.enter_context(tc.tile_pool(name="ppool_m", bufs=2, space="PSUM"))
        ppool_ct = ctx.enter_context(tc.tile_pool(name="ppool_ct", bufs=2, space="PSUM"))
        ppool_st = ctx.enter_context(tc.tile_pool(name="ppool_st", bufs=1, space="PSUM"))
        ppool_epi = ctx.enter_context(tc.tile_pool(name="ppool_epi", bufs=1, space="PSUM"))

        for g in range(Q // 2):
            i0g = g * 256
            shr_pair = []
            for q in range(2):
                I = 2 * g + q
                i0 = I * 128

                # distinct-column rel logits + exp (157 cols instead of 4224)
                ps_d = ppool_m.tile([128, S], F32, tag="ps_m")
                nc.tensor.matmul(
                    ps_d[:], qpT[:, ds(i0, 128)], rsT[:],
                    start=True, stop=True,
                )
                u_sb = wshear.tile([128, S], BF16, tag="u")
                nc.scalar.activation(out=u_sb[:], in_=ps_d[:], func=AF.Exp)

                # expand to the 4224-wide window on GPSIMD (per-element gather;
                # the segment index table is di-independent so per-core shared
                # index lists are exact). woff: window start in 16-col units.
                em_sb = wshear.tile([128, WN], BF16, tag="em")
                woff = (N - 128 - i0) // 16
                for c0 in range(0, WN, ic_chunk):
                    cw = min(ic_chunk, WN - c0)
                    nc.gpsimd.indirect_copy(
                        em_sb[:, ds(c0, cw)], u_sb[:],
                        widx_sb[:, ds(woff + c0 // 16, cw // 16)],
                        i_know_ap_gather_is_preferred=True,
                    )
                nc.sync.dma_start(out=em_d[ds(i0, 128), 0:WN - 1],
                                  in_=em_sb[:, 0:WN - 1])
                shr_sb = wshear.tile([128, N], BF16, tag="shr")
                shear_ap = bass.AP(em_d, i0 * WN + 127, [[WN - 1, 128], [1, N]])
                nc.sync.dma_start(out=shr_sb[:], in_=shear_ap)
                shr_pair.append(shr_sb)

            # content logits transposed: ecT[dj, J*256 + q*128 + di]
            ecT_sb = work.tile([128, NJ * 256], BF16, tag="ecT")
            for Jg in range(NJ // 2):
                ps = ppool_ct.tile([128, 512], F32, tag="ps_ct")
                for u in range(2):
                    J = Jg * 2 + u
                    nc.tensor.matmul(
                        ps[:, ts(u, 256)], kT[:, ts(J, 128)], qcT[:, ds(i0g, 256)],
                        start=True, stop=True,
                    )
                nc.scalar.activation(
                    out=ecT_sb[:, ds(Jg * 512, 512)], in_=ps[:], func=AF.Exp,
                )

            # pT = ecT * shr^T
            pT_sb = work.tile([128, NJ * 256], BF16, tag="pT")
            for Jg in range(NJ // 4):
                ps_t = ppool_st.tile([128, 1024], BF16, tag="ps_st")
                for u in range(4):
                    J = Jg * 4 + u
                    for q in range(2):
                        nc.tensor.transpose(
                            ps_t[:, ds(u * 256 + q * 128, 128)],
                            shr_pair[q][:, ts(J, 128)], ident[:],
                        )
                nc.vector.tensor_mul(
                    pT_sb[:, ds(Jg * 1024, 1024)], ecT_sb[:, ds(Jg * 1024, 1024)], ps_t[:]
                )

            # PV + epilogue per q-tile
            for q in range(2):
                i0 = i0g + q * 128
                ps_o = ppool_epi.tile([128, D + 1], F32, tag="ps_epi")
                for J in range(NJ):
                    nc.tensor.matmul(
                        ps_o[:], pT_sb[:, ds(J * 256 + q * 128, 128)],
                        vext[:, ds(J * (D + 1), D + 1)],
                        start=(J == 0), stop=(J == NJ - 1),
                    )
                rc_sb = sm.tile([128, 1], F32, tag="rc")
                nc.vector.reciprocal(out=rc_sb[:], in_=ps_o[:, D:D + 1])
                o_sb = sm.tile([128, D], BF16, tag="o")
                nc.vector.tensor_copy(o_sb[:], ps_o[:, 0:D])
                ps_ot = ppool_epi.tile([D, 128], BF16, tag="ps_epi")
                nc.tensor.transpose(ps_ot[:], o_sb[:], ident[:])
                otT_sb = sm.tile([D, 128], BF16, tag="otT")
                nc.vector.tensor_copy(otT_sb[:], ps_ot[:])
                out_sb = work.tile([128, DIM], FP16, tag="out")
                for w in range(DIM // 512):
                    ps_op = ppool_epi.tile([128, 512], F32, tag="ps_epi")
                    nc.tensor.matmul(
                        ps_op[:], otT_sb[:], wo_sb[:, ts(w, 512)],
                        start=True, stop=True,
                    )
                    nc.vector.tensor_scalar_mul(
                        out_sb[:, ts(w, 512)], ps_op[:], rc_sb[:]
                    )
                nc.sync.dma_start(out=out_d[ds(i0, 128), :], in_=out_sb[:])

    if split_waits:
        _patch_tile_drain()
        split_multi_waits(nc)
    return nc


# ---------------- host side ----------------

def get_positional_embed_np(seq_len, feature_size):
    distances = np.arange(-seq_len + 1, seq_len)
    nb = feature_size // 2
    pow_rate = math.exp(math.log(seq_len + 1) / nb)
    center_widths = np.power(np.float32(pow_rate), np.arange(1, nb + 1, dtype=np.float32)) - 1.0
    emb = (center_widths[None, :] > np.abs(distances)[:, None]).astype(np.float32)
    signed = np.sign(distances).astype(np.float32)[:, None] * emb
    return np.concatenate([emb, signed], axis=-1)  # [2n-1, F]


def make_in_maps(x, W_q, W_k, W_v, W_rel_k, W_out, rel_content_bias, rel_pos_bias):
    B, N, _ = np.asarray(x).shape
    WN = N + 128
    f16 = np.float16
    import ml_dtypes
    bf16 = ml_dtypes.bfloat16
    xT = np.ascontiguousarray(np.asarray(x[0], np.float32).T).astype(f16)
    pos = get_positional_embed_np(N, np.asarray(W_rel_k).shape[0])
    seg_starts, seg_of = pos_segments(N)
    S = len(seg_starts)
    pseg = np.ascontiguousarray(pos[seg_starts].T).astype(f16)  # [192, S]
    # wrapped master index table: window of q-tile I starts at rel index
    # N-128-128*I (multiple of 16), spans WN cols; idx list L = seg_of padded.
    L = np.zeros((2 * N - 1 + WN + 15) // 16 * 16, np.uint16)
    L[: 2 * N - 1] = seg_of.astype(np.uint16)
    WCOLS = len(L) // 16
    widx = np.zeros((128, WCOLS), np.uint16)
    Lw = L.reshape(WCOLS, 16).T  # [16, WCOLS]: Lw[w, s] = L[16s+w]
    for c in range(8):
        widx[16 * c:16 * (c + 1), :] = Lw
    in_maps = []
    for h in range(H):
        sl = slice(h * D, (h + 1) * D)
        in_maps.append({
            "xT": xT,
            "pseg": pseg,
            "widx": widx,
            "wq": np.ascontiguousarray(np.asarray(W_q)[:, sl]).astype(f16),
            "wk": np.ascontiguousarray(np.asarray(W_k)[:, sl]).astype(f16),
            "wv": np.ascontiguousarray(np.asarray(W_v)[:, sl]).astype(f16),
            "wrk": np.ascontiguousarray(np.asarray(W_rel_k)[:, sl]).astype(f16),
            "wo": np.ascontiguousarray(np.asarray(W_out)[sl, :]).astype(bf16),
            "bc": np.ascontiguousarray(
                np.asarray(rel_content_bias, np.float32)[0, h, 0, :].reshape(D, 1)),
            "bp": np.ascontiguousarray(
                np.asarray(rel_pos_bias, np.float32)[0, h, 0, :].reshape(D, 1)),
        })
    return in_maps


def combine_outputs(results, b_out):
    acc = None
    for r in results:
        p = r["out"].astype(np.float32)
        acc = p if acc is None else acc + p
    acc = acc + np.asarray(b_out, np.float32)[None, :]
    return acc[None]  # [1, N, DIM]


# ---------------- entry point ----------------

_NC_CACHE = {}


def kernel(x, W_q, W_k, W_v, W_rel_k, W_out, b_out,
           rel_content_bias, rel_pos_bias):
    """Full-input entry: shards per head across 8 NeuronCores, returns the
    full [1, N, 1536] float32 output."""
    from concourse import bass_utils

    x = np.asarray(x)
    N = x.shape[1]
    if N not in _NC_CACHE:
        _NC_CACHE[N] = build(N)
    nc = _NC_CACHE[N]
    in_maps = make_in_maps(x, W_q, W_k, W_v, W_rel_k, W_out,
                           rel_content_bias, rel_pos_bias)
    res = bass_utils.run_bass_kernel_spmd(nc, in_maps, core_ids=list(range(H)))
    return combine_outputs(res.results, b_out).astype(np.float32)



# revision 17
# speedup vs baseline: 1.2746x; 1.1792x over previous
"""Enformer-style relative-position attention (nn_Attention_27925877358942) for
8 Trainium2 NeuronCores.

Contract: kernel(**inputs) takes the FULL unsharded inputs (keys as in
setup_inputs()) and returns the full [1, 4096, 1536] float32 output.

Sharding: one head per core (8 heads / 8 cores). Host precomputes the
deterministic positional-feature table and x^T in fp16, slices per-head
weights, runs the SPMD Bass kernel via run_bass_kernel_spmd, and sums the
per-head output projections (+ b_out).

Device pipeline per core (head h), N=4096, d=64:
  - q^T,k^T (fp16, [64,N]) and v ([N,65] with ones col) projections on PE
  - r^T = (pos @ Wrelk_h)^T from the positional table
  - per query tile I: window logits em = exp((q+bp) . r[t0:t0+4223]) (ACT, bf16)
  - relative_shift via DRAM roundtrip: sheared strided read
      shr[di, j] = em[di, 127-di+j] (partition step = rowpitch-1 elements)
  - content logits transposed C^T = k_J . q_I (PE), exp on ACT
  - pT = exp(C^T) * transpose(shr) (PE transpose + DVE multiply, bf16)
  - O = pT.T @ [v|1] accumulated in PSUM; epilogue normalizes by the row sums
    and applies the per-head slice of W_out; host sums partials over heads.

This walrus build accepts at most ONE sync wait per instruction, so after
Tile scheduling every multi-wait instruction is split by inserting
wait-carrying NoOps just before it on the same engine (split_multi_waits),
and the Tile tail drain is built with the same constraint.
"""


_DRAIN_PATCHED = [False]


def _patch_tile_drain():
    if _DRAIN_PATCHED[0]:
        return
    _DRAIN_PATCHED[0] = True
    import concourse.tile as tile_mod
    from concourse.vector_clock import ScopedClock

    MAX_WAITS = 1

    def _drain_and_barrier(self, tick_clock, wait_clock):
        nc = self.nc
        drain_inst = nc.sync.drain()
        wait_clock.add_sem_waits(drain_inst.ins, ScopedClock({None: tick_clock.global_clock}))
        si = drain_inst.ins.sync_info
        waits = list(si.on_wait) if si is not None and si.on_wait else []
        if len(waits) > MAX_WAITS:
            si.on_wait = waits[:MAX_WAITS]
            rest = waits[MAX_WAITS:]
            import concourse.mybir as _mb
            for i in range(0, len(rest), MAX_WAITS):
                extra = nc.sync.drain()
                esi = extra.ins.sync_info
                if esi is None:
                    extra.ins.sync_info = _mb.SyncInfo(on_wait=rest[i:i + MAX_WAITS], on_update=[])
                else:
                    esi.on_wait = rest[i:i + MAX_WAITS]
        nc.all_engine_barrier()
        assert self.sems is not None
        popped = nc._tile_sem_poison_stack.pop()
        assert popped is self._sem_poison
        nc.clear_and_free_semaphores(list(self.sems.allocated().values()))
        nc.all_engine_barrier()

    tile_mod.TileContext._drain_and_barrier = _drain_and_barrier


def split_multi_waits(nc):
    """This walrus build allows at most ONE sync wait per instruction.
    Move extra waits onto InstNoOp carriers inserted just before, on the
    same engine queue (sequencers execute in order, so semantics hold)."""
    import concourse.mybir as mb
    n_split = 0
    for fn in nc.m.functions:
        for bb in fn.blocks:
            insts = list(bb.instructions)
            out = []
            for inst in insts:
                si = inst.sync_info
                waits = list(si.on_wait) if si is not None and si.on_wait else []
                if len(waits) > 1:
                    for w in waits[:-1]:
                        n_split += 1
                        nop = mb.InstNoOp(
                            name=f"waitsplit-{n_split}",
                            engine=inst.engine,
                            sync_info=mb.SyncInfo(on_wait=[w], on_update=[]),
                        )
                        out.append(nop)
                    si.on_wait = [waits[-1]]
                out.append(inst)
            if len(out) != len(insts):
                bb.instructions[:] = out
    return n_split


import math
from contextlib import ExitStack

import numpy as np

import concourse.bass as bass
import concourse.tile as tile
from concourse import mybir
from concourse.bass import ts, ds
from concourse.masks import make_identity

F32 = mybir.dt.float32
BF16 = mybir.dt.bfloat16
FP16 = mybir.dt.float16
U16 = mybir.dt.uint16
AF = mybir.ActivationFunctionType

DIM = 1536
H = 8
D = 64
NSEG = None  # set by build() from the static positional segmentation


def pos_segments(N):
    """Static segmentation of the 2N-1 relative positions: the central-mask
    features are piecewise constant in the distance, so the rel-k table has
    only ~157 distinct columns. Returns (seg_starts, seg_of)."""
    pos = get_positional_embed_np(N, 192)
    diffs = np.any(pos[1:] != pos[:-1], axis=1)
    seg_starts = np.concatenate([[0], np.nonzero(diffs)[0] + 1]).astype(np.int64)
    seg_of = np.zeros(2 * N - 1, np.int64)
    seg_of[seg_starts] = 1
    seg_of = np.cumsum(seg_of) - 1
    return seg_starts, seg_of


def build(N, split_waits=True, ic_chunk=1024):
    Q = N // 128           # query tiles
    NJ = N // 128          # key tiles
    WN = N + 128           # rel window width per q-tile (incl. 1 pad col)
    KD = DIM // 128        # contraction tiles for projections
    S = len(pos_segments(N)[0])  # distinct rel-k columns (157 for N=4096)
    WCOLS = (2 * N - 1 + WN + 15) // 16  # wrapped master index cols

    nc = bass.Bass("TRN2", target_bir_lowering=False, debug=False)

    xT_d = nc.dram_tensor("xT", [DIM, N], FP16, kind="ExternalInput")
    pseg_d = nc.dram_tensor("pseg", [192, S], FP16, kind="ExternalInput")
    widx_d = nc.dram_tensor("widx", [128, WCOLS], U16, kind="ExternalInput")
    wq_d = nc.dram_tensor("wq", [DIM, D], FP16, kind="ExternalInput")
    wk_d = nc.dram_tensor("wk", [DIM, D], FP16, kind="ExternalInput")
    wv_d = nc.dram_tensor("wv", [DIM, D], FP16, kind="ExternalInput")
    wrk_d = nc.dram_tensor("wrk", [192, D], FP16, kind="ExternalInput")
    wo_d = nc.dram_tensor("wo", [D, DIM], BF16, kind="ExternalInput")
    bc_d = nc.dram_tensor("bc", [D, 1], F32, kind="ExternalInput")
    bp_d = nc.dram_tensor("bp", [D, 1], F32, kind="ExternalInput")
    out_d = nc.dram_tensor("out", [N, DIM], FP16, kind="ExternalOutput")
    em_d = nc.dram_tensor("em_scratch", [Q * 128, WN], BF16, kind="Internal")

    scale = D ** -0.5

    with tile.TileContext(nc) as tc, ExitStack() as ctx:
        consts = ctx.enter_context(tc.tile_pool(name="consts", bufs=1))
        persist = ctx.enter_context(tc.tile_pool(name="persist", bufs=1))

        # ---- constants ----
        ident = consts.tile([128, 128], BF16, tag="ident")
        make_identity(nc, ident[:])
        bc_sb = consts.tile([D, 1], F32, tag="bc")
        nc.sync.dma_start(out=bc_sb[:], in_=bc_d.ap())
        bp_sb = consts.tile([D, 1], F32, tag="bp")
        nc.sync.dma_start(out=bp_sb[:], in_=bp_d.ap())
        wo_sb = consts.tile([D, DIM], BF16, tag="wo")
        nc.scalar.dma_start(out=wo_sb[:], in_=wo_d.ap())

        wqk_sb = consts.tile([128, KD, 2 * D], FP16, tag="wqk")
        wv_sb = consts.tile([128, KD, D], FP16, tag="wv")
        nc.scalar.dma_start(out=wqk_sb[:, :, 0:D],
                          in_=wq_d.ap().rearrange("(t p) c -> p t c", p=128))
        nc.scalar.dma_start(out=wqk_sb[:, :, D:2 * D],
                          in_=wk_d.ap().rearrange("(t p) c -> p t c", p=128))
        nc.scalar.dma_start(out=wv_sb[:],
                          in_=wv_d.ap().rearrange("(t p) c -> p t c", p=128))
        wrk_sb = consts.tile([96, 2, D], FP16, tag="wrk")
        for u in range(2):
            nc.sync.dma_start(out=wrk_sb[:, u, :], in_=wrk_d[ts(u, 96), :])

        # ---- persistent activations ----
        qcT = persist.tile([D, N], FP16, tag="qcT")
        qpT = persist.tile([D, N], FP16, tag="qpT")
        kT = persist.tile([D, N], FP16, tag="kT")
        rsT = persist.tile([D, S], FP16, tag="rsT")
        widx_sb = persist.tile([128, WCOLS], U16, tag="widx")
        vext = persist.tile([128, NJ * (D + 1)], BF16, tag="vext")

        nc.scalar.dma_start(out=widx_sb[:], in_=widx_d.ap())

        # ---- phases 1-2: rel-k table (distinct cols only) + projections ----
        with tc.tile_pool(name="stream", bufs=1) as stream, \
             tc.tile_pool(name="prep_psum", bufs=2, space="PSUM") as prep_psum:
            pall = stream.tile([96, 2, S], FP16, tag="pall")
            nc.scalar.dma_start(out=pall[:, 0, :], in_=pseg_d[0:96, :])
            nc.scalar.dma_start(out=pall[:, 1, :], in_=pseg_d[96:192, :])
            ps_r = prep_psum.tile([D, S], F32, tag="ps_qk")
            for u in range(2):
                nc.tensor.matmul(
                    ps_r[:], wrk_sb[:, u, :], pall[:, u, :],
                    start=(u == 0), stop=(u == 1),
                )
            nc.scalar.copy(out=rsT[:], in_=ps_r[:])

            xall = stream.tile([128, KD, N], FP16, tag="xall")
            xT_v = xT_d.ap().rearrange("(t p) n -> p t n", p=128)
            for oct_ in range(8):
                h0 = oct_ * (N // 8)
                nc.scalar.dma_start(
                    out=xall[:, :, ds(h0, N // 8)],
                    in_=xT_v[:, :, ds(h0, N // 8)],
                )
            for ic in range(N // 512):
                i0 = ic * 512
                xc = xall[:, :, ds(i0, 512)]
                ps_qk = prep_psum.tile([128, 512], F32, tag="ps_qk")
                for kd in range(KD):
                    nc.tensor.matmul(
                        ps_qk[:], wqk_sb[:, kd, :], xc[:, kd, :],
                        start=(kd == 0), stop=(kd == KD - 1),
                    )
                nc.scalar.activation(
                    out=qcT[:, ds(i0, 512)], in_=ps_qk[0:D, :], func=AF.Identity,
                    bias=bc_sb[:], scale=scale,
                )
                nc.scalar.activation(
                    out=qpT[:, ds(i0, 512)], in_=ps_qk[0:D, :], func=AF.Identity,
                    bias=bp_sb[:], scale=scale,
                )
                nc.scalar.copy(out=kT[:, ds(i0, 512)], in_=ps_qk[D:2 * D, :])
                for isb in range(4):
                    J = ic * 4 + isb
                    ps_v = prep_psum.tile([128, D], F32, tag="ps_v")
                    for kd in range(KD):
                        nc.tensor.matmul(
                            ps_v[:], xc[:, kd, ts(isb, 128)], wv_sb[:, kd, :],
                            start=(kd == 0), stop=(kd == KD - 1),
                        )
                    nc.scalar.copy(out=vext[:, ds(J * (D + 1), D)], in_=ps_v[:])
                    nc.vector.memset(vext[:, ds(J * (D + 1) + D, 1)], 1.0)

        # ---- phase 3: main loop, q-tiles in pairs, software-pipelined ----
        LAG = 1  # pairs of distance between shear production and consumption
        work = ctx.enter_context(tc.tile_pool(name="work", bufs=2))
        ecpool = ctx.enter_context(tc.tile_pool(name="ecpool", bufs=LAG + 2))
        upool = ctx.enter_context(tc.tile_pool(name="upool", bufs=4))
        empool = ctx.enter_context(tc.tile_pool(name="empool", bufs=3))
        shrpool = ctx.enter_context(tc.tile_pool(name="shrpool", bufs=2 * (LAG + 1) + 1))
        sm = ctx.enter_context(tc.tile_pool(name="sm", bufs=3))
        ppool_m = ctx.enter_context(tc.tile_pool(name="ppool_m", bufs=1, space="PSUM"))
        ppool_ct = ctx.enter_context(tc.tile_pool(name="ppool_ct", bufs=2, space="PSUM"))
        ppool_st = ctx.enter_context(tc.tile_pool(name="ppool_st", bufs=2, space="PSUM"))
        ppool_epi = ctx.enter_context(tc.tile_pool(name="ppool_epi", bufs=1, space="PSUM"))
        ppool_op = ctx.enter_context(tc.tile_pool(name="ppool_op", bufs=2, space="PSUM"))

        shr_live = {}
        ec_live = {}
        out_pending = []

        def flush_out():
            while out_pending:
                o_tile, oi0 = out_pending.pop(0)
                nc.scalar.dma_start(out=out_d[ds(oi0, 128), :], in_=o_tile[:])

        def produce(g):
            # Phase a: rel logits + expansion + em write for both tiles
            for q in range(2):
                I = 2 * g + q
                i0 = I * 128

                # distinct-column rel logits + exp (157 cols instead of 4224)
                ps_d = ppool_m.tile([128, S], F32, tag="ps_m")
                nc.tensor.matmul(
                    ps_d[:], qpT[:, ds(i0, 128)], rsT[:],
                    start=True, stop=True,
                )
                u_sb = upool.tile([128, S], BF16, tag="u")
                nc.scalar.activation(out=u_sb[:], in_=ps_d[:], func=AF.Exp)

                # expand to the 4224-wide window on GPSIMD (per-element gather;
                # the segment index table is di-independent so per-core shared
                # index lists are exact). woff: window start in 16-col units.
                em_sb = empool.tile([128, WN], BF16, tag="em")
                woff = (N - 128 - i0) // 16
                for c0 in range(0, WN, ic_chunk):
                    cw = min(ic_chunk, WN - c0)
                    nc.gpsimd.indirect_copy(
                        em_sb[:, ds(c0, cw)], u_sb[:],
                        widx_sb[:, ds(woff + c0 // 16, cw // 16)],
                        i_know_ap_gather_is_preferred=True,
                    )
                nc.sync.dma_start(out=em_d[ds(i0, 128), 0:WN - 1],
                                  in_=em_sb[:, 0:WN - 1])
            # Phase b: shear reads, issued after both em writes so their waits
            # are satisfied by the time they reach the head of the SP queue
            # (a DMA holds the issuing sequencer through wait+dge+transfer).
            shr_pair = []
            for q in range(2):
                i0 = (2 * g + q) * 128
                shr_sb = shrpool.tile([128, N], BF16, tag="shr")
                shear_ap = bass.AP(em_d, i0 * WN + 127, [[WN - 1, 128], [1, N]])
                nc.sync.dma_start(out=shr_sb[:], in_=shear_ap)
                shr_pair.append(shr_sb)
            shr_live[g] = shr_pair

            # content logits transposed: ecT[dj, J*256 + q*128 + di] --
            # produced here (independent of the shear) so the consume side
            # only has transposes/mul/PV left, shortening the drain tail.
            i0g = g * 256
            ecT_sb = ecpool.tile([128, NJ * 256], BF16, tag="ecT")
            for Jg in range(NJ // 2):
                ps = ppool_ct.tile([128, 512], F32, tag="ps_ct")
                for u in range(2):
                    J = Jg * 2 + u
                    nc.tensor.matmul(
                        ps[:, ts(u, 256)], kT[:, ts(J, 128)], qcT[:, ds(i0g, 256)],
                        start=True, stop=True,
                    )
                nc.scalar.activation(
                    out=ecT_sb[:, ds(Jg * 512, 512)], in_=ps[:], func=AF.Exp,
                )
            ec_live[g] = ecT_sb

        def consume(g):
            i0g = g * 256
            shr_pair = shr_live.pop(g)
            flush_out()

            ecT_sb = ec_live.pop(g)

            # pT = ecT * shr^T
            pT_sb = work.tile([128, NJ * 256], BF16, tag="pT")
            for Jg in range(NJ // 4):
                ps_t = ppool_st.tile([128, 1024], BF16, tag="ps_st")
                for u in range(4):
                    J = Jg * 4 + u
                    for q in range(2):
                        nc.tensor.transpose(
                            ps_t[:, ds(u * 256 + q * 128, 128)],
                            shr_pair[q][:, ts(J, 128)], ident[:],
                        )
                nc.vector.tensor_mul(
                    pT_sb[:, ds(Jg * 1024, 1024)], ecT_sb[:, ds(Jg * 1024, 1024)], ps_t[:]
                )

            # PV + epilogue per q-tile
            for q in range(2):
                i0 = i0g + q * 128
                ps_o = ppool_epi.tile([128, 512], F32, tag="ps_o")
                for J in range(NJ):
                    nc.tensor.matmul(
                        ps_o[:, 0:D + 1], pT_sb[:, ds(J * 256 + q * 128, 128)],
                        vext[:, ds(J * (D + 1), D + 1)],
                        start=(J == 0), stop=(J == NJ - 1),
                    )
                rc_sb = sm.tile([128, 1], F32, tag="rc")
                nc.vector.reciprocal(out=rc_sb[:], in_=ps_o[:, D:D + 1])
                o_sb = sm.tile([128, D], BF16, tag="o")
                nc.vector.tensor_copy(o_sb[:], ps_o[:, 0:D])
                ps_ot = ps_o[0:D, 128:192].bitcast(BF16)
                nc.tensor.transpose(ps_ot, o_sb[:], ident[:])
                otT_sb = sm.tile([D, 128], BF16, tag="otT")
                nc.vector.tensor_copy(otT_sb[:], ps_ot)
                out_sb = work.tile([128, DIM], FP16, tag="out")
                for w in range(DIM // 512):
                    ps_op = ppool_op.tile([128, 512], F32, tag="ps_op")
                    nc.tensor.matmul(
                        ps_op[:], otT_sb[:], wo_sb[:, ts(w, 512)],
                        start=True, stop=True,
                    )
                    nc.vector.tensor_scalar_mul(
                        out_sb[:, ts(w, 512)], ps_op[:], rc_sb[:]
                    )
                # out-write deferred one pair and issued from the ACT queue:
                # by then its producer (DVE scale) has finished, so the DMA
                # holds ACT.SEQ only for dge+transfer, and the SP queue stays
                # dedicated to the em/shear stream.
                out_pending.append((out_sb, i0))

        for g in range(Q // 2 + LAG):
            if g < Q // 2:
                produce(g)
            if g >= LAG:
                consume(g - LAG)
        flush_out()

    if split_waits:
        _patch_tile_drain()
        split_multi_waits(nc)
    return nc


# ---------------- host side ----------------

def get_positional_embed_np(seq_len, feature_size):
    distances = np.arange(-seq_len + 1, seq_len)
    nb = feature_size // 2
    pow_rate = math.exp(math.log(seq_len + 1) / nb)
    center_widths = np.power(np.float32(pow_rate), np.arange(1, nb + 1, dtype=np.float32)) - 1.0
    emb = (center_widths[None, :] > np.abs(distances)[:, None]).astype(np.float32)
    signed = np.sign(distances).astype(np.float32)[:, None] * emb
    return np.concatenate([emb, signed], axis=-1)  # [2n-1, F]


def make_in_maps(x, W_q, W_k, W_v, W_rel_k, W_out, rel_content_bias, rel_pos_bias):
    B, N, _ = np.asarray(x).shape
    WN = N + 128
    f16 = np.float16
    import ml_dtypes
    bf16 = ml_dtypes.bfloat16
    xT = np.ascontiguousarray(np.asarray(x[0], np.float32).T).astype(f16)
    pos = get_positional_embed_np(N, np.asarray(W_rel_k).shape[0])
    seg_starts, seg_of = pos_segments(N)
    S = len(seg_starts)
    pseg = np.ascontiguousarray(pos[seg_starts].T).astype(f16)  # [192, S]
    # wrapped master index table: window of q-tile I starts at rel index
    # N-128-128*I (multiple of 16), spans WN cols; idx list L = seg_of padded.
    L = np.zeros((2 * N - 1 + WN + 15) // 16 * 16, np.uint16)
    L[: 2 * N - 1] = seg_of.astype(np.uint16)
    WCOLS = len(L) // 16
    widx = np.zeros((128, WCOLS), np.uint16)
    Lw = L.reshape(WCOLS, 16).T  # [16, WCOLS]: Lw[w, s] = L[16s+w]
    for c in range(8):
        widx[16 * c:16 * (c + 1), :] = Lw
    in_maps = []
    for h in range(H):
        sl = slice(h * D, (h + 1) * D)
        in_maps.append({
            "xT": xT,
            "pseg": pseg,
            "widx": widx,
            "wq": np.ascontiguousarray(np.asarray(W_q)[:, sl]).astype(f16),
            "wk": np.ascontiguousarray(np.asarray(W_k)[:, sl]).astype(f16),
            "wv": np.ascontiguousarray(np.asarray(W_v)[:, sl]).astype(f16),
            "wrk": np.ascontiguousarray(np.asarray(W_rel_k)[:, sl]).astype(f16),
            "wo": np.ascontiguousarray(np.asarray(W_out)[sl, :]).astype(bf16),
            "bc": np.ascontiguousarray(
                np.asarray(rel_content_bias, np.float32)[0, h, 0, :].reshape(D, 1)),
            "bp": np.ascontiguousarray(
                np.asarray(rel_pos_bias, np.float32)[0, h, 0, :].reshape(D, 1)),
        })
    return in_maps


def combine_outputs(results, b_out):
    acc = None
    for r in results:
        p = r["out"].astype(np.float32)
        acc = p if acc is None else acc + p
    acc = acc + np.asarray(b_out, np.float32)[None, :]
    return acc[None]  # [1, N, DIM]


# ---------------- entry point ----------------

_NC_CACHE = {}


def kernel(x, W_q, W_k, W_v, W_rel_k, W_out, b_out,
           rel_content_bias, rel_pos_bias):
    """Full-input entry: shards per head across 8 NeuronCores, returns the
    full [1, N, 1536] float32 output."""
    from concourse import bass_utils

    x = np.asarray(x)
    N = x.shape[1]
    if N not in _NC_CACHE:
        _NC_CACHE[N] = build(N)
    nc = _NC_CACHE[N]
    in_maps = make_in_maps(x, W_q, W_k, W_v, W_rel_k, W_out,
                           rel_content_bias, rel_pos_bias)
    res = bass_utils.run_bass_kernel_spmd(nc, in_maps, core_ids=list(range(H)))
    return combine_outputs(res.results, b_out).astype(np.float32)

